# revision 7
# baseline (speedup 1.0000x reference)
"""Trainium2 Bass kernel for nn_Convolution_61993557950960.

Equivariant 5x5x5 conv: x (2,64,40,40,40) f32 -> out (2,144,40,40,40) f32.

Strategy (8 NeuronCores, data-parallel over (batch, D)):
  - Host: build the effective OIDHW conv kernel from (w_tp, w_sc0, w_sc1)
    (tiny einsums, replicated), folding the self-connection into the center
    tap so that out = 0.1 * correlate3d(x, kernel_eff, pad=2).
  - Each core gets 10 output D-slices (+2 halo each side) as a zero-padded
    bf16 tensor [64ch, 14*44*44] and computes the conv as an implicit GEMM:
    contraction K = 64ch x 2 taps (tap pairing via a second, shifted copy of
    x living in SBUF partitions 64..127), moving operand = conv weights
    [K, Cout-chunk], stationary = x-patch view, N = up to 480 voxels.
  - Cout=144 split: main M=128 (couts 0..127) + remainder M=16 (couts
    128..143) executed 4-way col-tiled (tile_position) so 4 remainder
    matmuls run concurrently in distinct 32-column groups of the PE array.
"""

import os
import numpy as np
import ml_dtypes

# ----------------------------------------------------------------------------
# geometry (hardcoded for this problem)
# ----------------------------------------------------------------------------
SIZE = 5
MUL = 16
DIM_IN = 4 * MUL        # 64
DIM_OUT = 9 * MUL       # 144
BATCH = 2
S = 40                  # spatial
NCORES = 8
SLABS = 10              # output D-slices per core
DP = SLABS + 4          # 14 padded input slices per core
HP = 44
WP = 44
ROWS = DP * HP          # 616
LFLAT = ROWS * WP       # 27104
SLICE = HP * WP         # 1936
LSTORE = LFLAT + SLICE + 8   # tail so the +1 / +SLICE shifted loads stay in bounds
H_TILES = ((0, 12), (12, 12), (24, 12), (36, 4))
REM_Q = 4               # remainder col-groups, 10 H-rows each

SQ3 = float(np.sqrt(3.0))
SQ5 = float(np.sqrt(5.0))
INV_SQRT_MUL = float(1.0 / np.sqrt(MUL))
PW0 = float(np.sqrt(1.0 / 32.0))
PW1 = float(np.sqrt(3.0 / 48.0))
PW2 = float(np.sqrt(5.0 / 32.0))

BF16 = ml_dtypes.bfloat16


def _make_pairs():
    """Tap pairing. Each entry: (sel, a, b, c, K) with t1=(a,b,c).

    sel 0 -> tensor A (partitions 64..127 hold x shifted +1 elem: pair
             (a,b,c)+(a,b,c+1));
    sel 1 -> tensor B (shift +1936 = one D-slice: pair (a,b,c)+(a+1,b,c));
    sel 2 -> tensor C (shift +44 = one H-row: pair (a,b,c)+(a,b+1,c));
    K=64 -> unpaired single tap (uses partitions 0..63 only).
    """
    pairs = []
    for a in range(5):
        for b in range(5):
            for c in (0, 2):
                pairs.append((0, a, b, c, 128))
    for a in (0, 2):
        for b in range(5):
            pairs.append((1, a, b, 4, 128))
    for b in (0, 2):
        pairs.append((2, 4, b, 4, 128))
    pairs.append((0, 4, 4, 4, 64))
    return pairs


PAIRS = _make_pairs()
NPAIR = len(PAIRS)      # 65

# ----------------------------------------------------------------------------
# host-side kernel generation (the tiny einsums; replicated on every core)
# ----------------------------------------------------------------------------

def _sus(x):
    return np.where(x > 0.0, np.exp(-1.0 / np.where(x > 0.0, x, 1.0)), 0.0)


def _buffers(dtype=np.float64):
    r = np.linspace(-1.0, 1.0, SIZE)
    X, Y, Z = np.meshgrid(r, r, r, indexing='ij')
    lattice = np.stack([X, Y, Z], axis=-1)
    d = np.linalg.norm(lattice, axis=-1)
    values = np.linspace(0.0, 1.0, SIZE + 2)[1:-1]
    step = values[1] - values[0]
    diff = (d[..., None] - values) / step
    emb = 1.14136 * np.exp(2.0) * _sus(diff + 1.0) * _sus(1.0 - diff)
    n = np.maximum(d, 1e-12)[..., None]
    u = lattice / n
    x, y, z = u[..., 0], u[..., 1], u[..., 2]
    sh0 = np.ones_like(x)[..., None]
    sh1 = SQ3 * u
    sh2 = SQ5 * np.stack([
        SQ3 * x * z,
        SQ3 * x * y,
        y ** 2 - 0.5 * (x ** 2 + z ** 2),
        SQ3 * y * z,
        0.5 * SQ3 * (z ** 2 - x ** 2),
    ], axis=-1)
    sh = np.concatenate([sh0, sh1, sh2], axis=-1)
    return emb.astype(dtype), sh.astype(dtype)


def _w3j_112(dtype=np.float64):
    s = np.sqrt(2.0 / 15.0)
    h = 0.5 * np.sqrt(3.0) * s
    C = np.zeros((3, 3, 5), dtype=dtype)
    C[0, 2, 0] = C[2, 0, 0] = h
    C[0, 1, 1] = C[1, 0, 1] = h
    C[0, 0, 2] = -0.5 * s; C[1, 1, 2] = s; C[2, 2, 2] = -0.5 * s
    C[1, 2, 3] = C[2, 1, 3] = h
    C[0, 0, 4] = -h; C[2, 2, 4] = h
    return C


def build_kernel_eff(w_sc0, w_sc1, w_tp, dtype=np.float64):
    """kernel_eff [144, 64, 5,5,5]: out = 0.1*correlate3d(x, kernel_eff, pad=2)."""
    w_sc0 = np.asarray(w_sc0, dtype)
    w_sc1 = np.asarray(w_sc1, dtype)
    w_tp = np.asarray(w_tp, dtype)
    emb, sh = _buffers(dtype)
    C112 = _w3j_112(dtype)
    C121 = np.transpose(C112, (0, 2, 1))
    Sz = SIZE
    sh0, sh1, sh2 = sh[..., 0:1], sh[..., 1:4], sh[..., 4:9]
    w = np.einsum('xyzr,rn->xyzn', emb, w_tp) / Sz ** 1.5
    n2 = MUL * MUL
    W = [w[..., i * n2:(i + 1) * n2].reshape(Sz, Sz, Sz, MUL, MUL) for i in range(7)]
    eye3 = np.eye(3, dtype=dtype)
    B00 = PW0 * sh0[..., None] * W[0]
    B01 = (PW1 / SQ3) * np.einsum('xyzk,xyzuw->xyzuwk', sh1, W[1]).reshape(Sz, Sz, Sz, MUL, 3 * MUL)
    B02 = (PW2 / SQ5) * np.einsum('xyzk,xyzuw->xyzuwk', sh2, W[2]).reshape(Sz, Sz, Sz, MUL, 5 * MUL)
    B10 = (PW0 / SQ3) * np.einsum('xyzi,xyzuw->xyzuiw', sh1, W[4]).reshape(Sz, Sz, Sz, 3 * MUL, MUL)
    B11 = ((PW1 / SQ3) * np.einsum('xyz,xyzuw,ik->xyzuiwk', sh0[..., 0], W[3], eye3)
           + PW1 * np.einsum('ijk,xyzj,xyzuw->xyzuiwk', C121, sh2, W[6])
           ).reshape(Sz, Sz, Sz, 3 * MUL, 3 * MUL)
    B12 = PW2 * np.einsum('ijk,xyzj,xyzuw->xyzuiwk', C112, sh1, W[5]).reshape(Sz, Sz, Sz, 3 * MUL, 5 * MUL)
    K = np.concatenate([np.concatenate([B00, B01, B02], axis=-1),
                        np.concatenate([B10, B11, B12], axis=-1)], axis=-2)
    kernel = np.transpose(K, (4, 3, 0, 1, 2)).copy()

    # fold self-connection into the center tap: out = 0.1*(conv + 10*sc)
    Wsc = np.zeros((DIM_IN, DIM_OUT), dtype)
    Wsc[:MUL, :MUL] = w_sc0 * INV_SQRT_MUL
    w1 = w_sc1 * INV_SQRT_MUL
    for i3 in range(3):
        Wsc[MUL + i3:MUL + 48:3, MUL + i3:MUL + 48:3] += w1
    kernel[:, :, 2, 2, 2] += 10.0 * Wsc.T
    return kernel


def pack_weights(kernel_eff):
    """Pack per-pair stationary weight blocks -> (wmain [128, NPAIR*128],
    wrem [128, NPAIR*16]) bf16."""
    Wt = kernel_eff.transpose(1, 0, 2, 3, 4)  # [in, out, a, b, c]
    wmain = np.zeros((128, NPAIR, 128), np.float64)
    wrem = np.zeros((128, NPAIR, 16), np.float64)
    for p, (sel, a, b, c, K) in enumerate(PAIRS):
        W1 = Wt[:, :, a, b, c]
        wmain[0:64, p, :] = W1[:, 0:128]
        wrem[0:64, p, :] = W1[:, 128:144]
        if K == 128:
            if sel == 0:
                a2, b2, c2 = a, b, c + 1
            elif sel == 1:
                a2, b2, c2 = a + 1, b, c
            else:
                a2, b2, c2 = a, b + 1, c
            W2 = Wt[:, :, a2, b2, c2]
            wmain[64:128, p, :] = W2[:, 0:128]
            wrem[64:128, p, :] = W2[:, 128:144]
    return (wmain.astype(BF16).reshape(128, -1),
            wrem.astype(BF16).reshape(128, -1))


def shard_x(x):
    """x (2,64,40,40,40) f32 -> per-core padded bf16 [64, LSTORE]."""
    xbf = np.ascontiguousarray(x).astype(BF16)
    shards = []
    for core in range(NCORES):
        b, chunk = divmod(core, 4)
        d0 = chunk * SLABS
        xp = np.zeros((64, DP, HP, WP), BF16)
        lo = max(0, d0 - 2)
        hi = min(S, d0 + SLABS + 2)
        xp[:, lo - (d0 - 2):hi - (d0 - 2), 2:2 + S, 2:2 + S] = xbf[b, :, lo:hi]
        flat = np.zeros((64, LSTORE), BF16)
        flat[:, :LFLAT] = xp.reshape(64, LFLAT)
        shards.append(flat)
    return shards


# ----------------------------------------------------------------------------
# bass program (built & compiled once per process)
# ----------------------------------------------------------------------------
_PROG = None


def _build_program():
    import concourse.bass as bass  # noqa: F401
    import concourse.bacc as bacc
    import concourse.mybir as mybir
    import concourse.tile as tile

    nc = bacc.Bacc("TRN2", target_bir_lowering=False, debug=False,
                   enable_asserts=False, num_devices=NCORES)
    bf = mybir.dt.bfloat16
    f32 = mybir.dt.float32

    xpad = nc.dram_tensor("xpad", [64, LSTORE], bf, kind="ExternalInput").ap()
    wmain_d = nc.dram_tensor("wmain", [128, NPAIR * 128], bf, kind="ExternalInput").ap()
    wrem_d = nc.dram_tensor("wrem", [128, NPAIR * 16], bf, kind="ExternalInput").ap()
    out_d = nc.dram_tensor("out", [DIM_OUT, SLABS, S, S], f32, kind="ExternalOutput").ap()
    warm_d = nc.dram_tensor("warm", [128, 16], f32, kind="ExternalOutput").ap()

    with tile.TileContext(nc) as tc:
        with tc.tile_pool(name="xpool", bufs=1) as xpool, \
             tc.tile_pool(name="wpool", bufs=1) as wpool, \
             tc.tile_pool(name="opool", bufs=3) as opool, \
             tc.tile_pool(name="pmain", bufs=3, space="PSUM") as pmain_pool, \
             tc.tile_pool(name="prem", bufs=2, space="PSUM") as prem_pool:

            # HAM warm-up: ~4us of dependency-free matmuls on zeroed scratch
            # run during the initial DMA fill (PE is otherwise idle), so the
            # real matmul stream starts at 2.4 GHz instead of 1.2 GHz.
            wscr = wpool.tile([128, 640], bf)
            nc.vector.memset(wscr, 0)
            pwarm = pmain_pool.tile([128, 512], f32, tag="pwarm", bufs=1)
            for _ in range(10):
                nc.tensor.matmul(pwarm, wscr[:, 0:128], wscr[:, 128:640],
                                 start=True, stop=True)
            wout = opool.tile([128, 16], f32, tag="wout", bufs=1)
            nc.vector.tensor_scalar_mul(wout, pwarm[:, 0:16], 1.0)
            nc.sync.dma_start(out=warm_d, in_=wout)

            xA = xpool.tile([128, ROWS, WP], bf)
            xB = xpool.tile([128, ROWS, WP], bf)
            xC = xpool.tile([128, ROWS, WP], bf)
            Wm = wpool.tile([128, NPAIR, 128], bf)
            Wr = wpool.tile([128, NPAIR, 16], bf)

            Wmf = Wm.rearrange("p a b -> p (a b)")

            xAf = xA.rearrange("p r c -> p (r c)")
            xBf = xB.rearrange("p r c -> p (r c)")
            xCf = xC.rearrange("p r c -> p (r c)")
            XTENS = (xA, xB, xC)

            def load_slice(xf, s, shift):
                sl = slice(s * SLICE, (s + 1) * SLICE)
                nc.sync.dma_start(out=xf[0:64, sl], in_=xpad[:, sl])
                nc.sync.dma_start(out=xf[64:128, sl],
                                  in_=xpad[:, s * SLICE + shift:(s + 1) * SLICE + shift])

            def load_w_chunk(p0, p1):
                nc.sync.dma_start(out=Wmf[:, p0 * 128:p1 * 128],
                                  in_=wmain_d[:, p0 * 128:p1 * 128])

            # need-ordered loads: slab-0 critical set first, weight DMA
            # chunked so the first matmuls (A-pairs with a=0, emitted first
            # in each accumulation group) can start after ~1MB of DMA.
            # used dst slices: A 0..13, B 0..11, C 4..13.
            load_w_chunk(0, 16)
            load_slice(xAf, 0, 1)
            load_w_chunk(16, 32)
            load_slice(xAf, 1, 1)
            load_w_chunk(32, 48)
            load_slice(xAf, 2, 1)
            load_w_chunk(48, NPAIR)
            for s in (3, 4):
                load_slice(xAf, s, 1)
            nc.sync.dma_start(out=Wr.rearrange("p a b -> p (a b)"), in_=wrem_d)
            for s in range(4):
                load_slice(xBf, s, SLICE)
            load_slice(xCf, 4, HP)
            for k in range(1, SLABS):
                load_slice(xAf, k + 4, 1)
                if k + 3 < 12:
                    load_slice(xBf, k + 3, SLICE)
                load_slice(xCf, k + 4, HP)

            def emit_main_tile(d, h0, nr):
                pm = pmain_pool.tile([128, nr, S], f32, tag="pm")
                for i, (sel, a, b, c, K) in enumerate(PAIRS):
                    r0 = (d + a) * HP + h0 + b
                    rhs = XTENS[sel][0:K, r0:r0 + nr, c:c + S]
                    nc.tensor.matmul(pm, Wm[0:K, i, :], rhs,
                                     start=(i == 0), stop=(i == NPAIR - 1))
                ot = opool.tile([128, nr, S], f32, tag="ot")
                nc.vector.tensor_scalar_mul(ot, pm, 0.1)
                nc.sync.dma_start(out=out_d[0:128, d, h0:h0 + nr, :], in_=ot)

            def emit_rem(d):
                # remainder couts 128..143, 4-way col-tiled (10 H-rows/group)
                pr = prem_pool.tile([128, 10, S], f32, tag="pr")
                for i, (sel, a, b, c, K) in enumerate(PAIRS):
                    for q in range(REM_Q):
                        r0 = (d + a) * HP + 10 * q + b
                        rhs = XTENS[sel][0:K, r0:r0 + 10, c:c + S]
                        nc.tensor.matmul(pr[32 * q:32 * q + 16, :, :],
                                         Wr[0:K, i, :], rhs,
                                         start=(i == 0), stop=(i == NPAIR - 1),
                                         tile_position=(0, 32 * q))
                orem = opool.tile([128, 10, S], f32, tag="orem")
                for q in range(REM_Q):
                    nc.vector.tensor_scalar_mul(orem[32 * q:32 * q + 16],
                                                pr[32 * q:32 * q + 16], 0.1)
                    nc.sync.dma_start(out=out_d[128:144, d, 10 * q:10 * q + 10, :],
                                      in_=orem[32 * q:32 * q + 16])

            for d in range(SLABS):
                for h0, nr in H_TILES[:-1]:
                    emit_main_tile(d, h0, nr)
                emit_rem(d)
                h0, nr = H_TILES[-1]
                emit_main_tile(d, h0, nr)

    nc.compile()
    return nc


def _get_program():
    global _PROG
    if _PROG is None:
        _PROG = _build_program()
    return _PROG


# ----------------------------------------------------------------------------
# entry points
# ----------------------------------------------------------------------------

def run_on_hw(inputs, trace=False, tmpdir=None):
    from concourse.bass_utils import run_bass_kernel_spmd

    nc = _get_program()
    ker = build_kernel_eff(inputs['w_sc0'], inputs['w_sc1'], inputs['w_tp'])
    wmain, wrem = pack_weights(ker)
    shards = shard_x(np.asarray(inputs['x']))
    in_maps = [{"xpad": shards[c], "wmain": wmain, "wrem": wrem}
               for c in range(NCORES)]
    res = run_bass_kernel_spmd(nc, in_maps, list(range(NCORES)),
                               trace=trace, tmpdir=tmpdir)
    out = np.zeros((BATCH, DIM_OUT, S, S, S), np.float32)
    for core in range(NCORES):
        b, chunk = divmod(core, 4)
        d0 = chunk * SLABS
        out[b, :, d0:d0 + SLABS] = res.results[core]["out"]
    return out, res


def kernel(x, w_sc0, w_sc1, w_tp):
    out, _ = run_on_hw({'x': x, 'w_sc0': w_sc0, 'w_sc1': w_sc1, 'w_tp': w_tp})
    return out


# revision 9
# speedup vs baseline: 1.0090x; 1.0090x over previous
"""Trainium2 Bass kernel for nn_Convolution_61993557950960.

Equivariant 5x5x5 conv: x (2,64,40,40,40) f32 -> out (2,144,40,40,40) f32.

Strategy (8 NeuronCores, data-parallel over (batch, D)):
  - Host: build the effective OIDHW conv kernel from (w_tp, w_sc0, w_sc1)
    (tiny einsums, replicated), folding the self-connection into the center
    tap so that out = 0.1 * correlate3d(x, kernel_eff, pad=2).
  - Each core gets 10 output D-slices (+2 halo each side) as a zero-padded
    bf16 tensor [64ch, 14*44*44] and computes the conv as an implicit GEMM:
    contraction K = 64ch x 2 taps (tap pairing via a second, shifted copy of
    x living in SBUF partitions 64..127), moving operand = conv weights
    [K, Cout-chunk], stationary = x-patch view, N = up to 480 voxels.
  - Cout=144 split: main M=128 (couts 0..127) + remainder M=16 (couts
    128..143) executed 4-way col-tiled (tile_position) so 4 remainder
    matmuls run concurrently in distinct 32-column groups of the PE array.
"""

import os
import numpy as np
import ml_dtypes

# ----------------------------------------------------------------------------
# geometry (hardcoded for this problem)
# ----------------------------------------------------------------------------
SIZE = 5
MUL = 16
DIM_IN = 4 * MUL        # 64
DIM_OUT = 9 * MUL       # 144
BATCH = 2
S = 40                  # spatial
NCORES = 8
SLABS = 10              # output D-slices per core
DP = SLABS + 4          # 14 padded input slices per core
HP = 44
WP = 44
ROWS = DP * HP          # 616
LFLAT = ROWS * WP       # 27104
SLICE = HP * WP         # 1936
LSTORE = LFLAT + SLICE + 8   # tail so the +1 / +SLICE shifted loads stay in bounds
H_TILES = ((0, 12), (12, 12), (24, 12), (36, 4))
REM_Q = 4               # remainder col-groups, 10 H-rows each

SQ3 = float(np.sqrt(3.0))
SQ5 = float(np.sqrt(5.0))
INV_SQRT_MUL = float(1.0 / np.sqrt(MUL))
PW0 = float(np.sqrt(1.0 / 32.0))
PW1 = float(np.sqrt(3.0 / 48.0))
PW2 = float(np.sqrt(5.0 / 32.0))

BF16 = ml_dtypes.bfloat16


def _make_pairs():
    """Tap pairing. Each entry: (sel, a, b, c, K) with t1=(a,b,c).

    sel 0 -> tensor A (partitions 64..127 hold x shifted +1 elem: pair
             (a,b,c)+(a,b,c+1));
    sel 1 -> tensor B (shift +1936 = one D-slice: pair (a,b,c)+(a+1,b,c));
    sel 2 -> tensor C (shift +44 = one H-row: pair (a,b,c)+(a,b+1,c));
    K=64 -> unpaired single tap (uses partitions 0..63 only).
    """
    pairs = []
    for a in range(5):
        for b in range(5):
            for c in (0, 2):
                pairs.append((0, a, b, c, 128))
    for a in (0, 2):
        for b in range(5):
            pairs.append((1, a, b, 4, 128))
    for b in (0, 2):
        pairs.append((2, 4, b, 4, 128))
    pairs.append((0, 4, 4, 4, 64))
    return pairs


PAIRS = _make_pairs()
NPAIR = len(PAIRS)      # 65

# ----------------------------------------------------------------------------
# host-side kernel generation (the tiny einsums; replicated on every core)
# ----------------------------------------------------------------------------

def _sus(x):
    return np.where(x > 0.0, np.exp(-1.0 / np.where(x > 0.0, x, 1.0)), 0.0)


def _buffers(dtype=np.float64):
    r = np.linspace(-1.0, 1.0, SIZE)
    X, Y, Z = np.meshgrid(r, r, r, indexing='ij')
    lattice = np.stack([X, Y, Z], axis=-1)
    d = np.linalg.norm(lattice, axis=-1)
    values = np.linspace(0.0, 1.0, SIZE + 2)[1:-1]
    step = values[1] - values[0]
    diff = (d[..., None] - values) / step
    emb = 1.14136 * np.exp(2.0) * _sus(diff + 1.0) * _sus(1.0 - diff)
    n = np.maximum(d, 1e-12)[..., None]
    u = lattice / n
    x, y, z = u[..., 0], u[..., 1], u[..., 2]
    sh0 = np.ones_like(x)[..., None]
    sh1 = SQ3 * u
    sh2 = SQ5 * np.stack([
        SQ3 * x * z,
        SQ3 * x * y,
        y ** 2 - 0.5 * (x ** 2 + z ** 2),
        SQ3 * y * z,
        0.5 * SQ3 * (z ** 2 - x ** 2),
    ], axis=-1)
    sh = np.concatenate([sh0, sh1, sh2], axis=-1)
    return emb.astype(dtype), sh.astype(dtype)


def _w3j_112(dtype=np.float64):
    s = np.sqrt(2.0 / 15.0)
    h = 0.5 * np.sqrt(3.0) * s
    C = np.zeros((3, 3, 5), dtype=dtype)
    C[0, 2, 0] = C[2, 0, 0] = h
    C[0, 1, 1] = C[1, 0, 1] = h
    C[0, 0, 2] = -0.5 * s; C[1, 1, 2] = s; C[2, 2, 2] = -0.5 * s
    C[1, 2, 3] = C[2, 1, 3] = h
    C[0, 0, 4] = -h; C[2, 2, 4] = h
    return C


def build_kernel_eff(w_sc0, w_sc1, w_tp, dtype=np.float64):
    """kernel_eff [144, 64, 5,5,5]: out = 0.1*correlate3d(x, kernel_eff, pad=2)."""
    w_sc0 = np.asarray(w_sc0, dtype)
    w_sc1 = np.asarray(w_sc1, dtype)
    w_tp = np.asarray(w_tp, dtype)
    emb, sh = _buffers(dtype)
    C112 = _w3j_112(dtype)
    C121 = np.transpose(C112, (0, 2, 1))
    Sz = SIZE
    sh0, sh1, sh2 = sh[..., 0:1], sh[..., 1:4], sh[..., 4:9]
    w = np.einsum('xyzr,rn->xyzn', emb, w_tp) / Sz ** 1.5
    n2 = MUL * MUL
    W = [w[..., i * n2:(i + 1) * n2].reshape(Sz, Sz, Sz, MUL, MUL) for i in range(7)]
    eye3 = np.eye(3, dtype=dtype)
    B00 = PW0 * sh0[..., None] * W[0]
    B01 = (PW1 / SQ3) * np.einsum('xyzk,xyzuw->xyzuwk', sh1, W[1]).reshape(Sz, Sz, Sz, MUL, 3 * MUL)
    B02 = (PW2 / SQ5) * np.einsum('xyzk,xyzuw->xyzuwk', sh2, W[2]).reshape(Sz, Sz, Sz, MUL, 5 * MUL)
    B10 = (PW0 / SQ3) * np.einsum('xyzi,xyzuw->xyzuiw', sh1, W[4]).reshape(Sz, Sz, Sz, 3 * MUL, MUL)
    B11 = ((PW1 / SQ3) * np.einsum('xyz,xyzuw,ik->xyzuiwk', sh0[..., 0], W[3], eye3)
           + PW1 * np.einsum('ijk,xyzj,xyzuw->xyzuiwk', C121, sh2, W[6])
           ).reshape(Sz, Sz, Sz, 3 * MUL, 3 * MUL)
    B12 = PW2 * np.einsum('ijk,xyzj,xyzuw->xyzuiwk', C112, sh1, W[5]).reshape(Sz, Sz, Sz, 3 * MUL, 5 * MUL)
    K = np.concatenate([np.concatenate([B00, B01, B02], axis=-1),
                        np.concatenate([B10, B11, B12], axis=-1)], axis=-2)
    kernel = np.transpose(K, (4, 3, 0, 1, 2)).copy()

    # fold self-connection into the center tap: out = 0.1*(conv + 10*sc)
    Wsc = np.zeros((DIM_IN, DIM_OUT), dtype)
    Wsc[:MUL, :MUL] = w_sc0 * INV_SQRT_MUL
    w1 = w_sc1 * INV_SQRT_MUL
    for i3 in range(3):
        Wsc[MUL + i3:MUL + 48:3, MUL + i3:MUL + 48:3] += w1
    kernel[:, :, 2, 2, 2] += 10.0 * Wsc.T
    return kernel


def pack_weights(kernel_eff):
    """Pack per-pair stationary weight blocks -> (wmain [128, NPAIR*128],
    wrem [128, NPAIR*16]) bf16."""
    Wt = kernel_eff.transpose(1, 0, 2, 3, 4)  # [in, out, a, b, c]
    wmain = np.zeros((128, NPAIR, 128), np.float64)
    wrem = np.zeros((128, NPAIR, 16), np.float64)
    for p, (sel, a, b, c, K) in enumerate(PAIRS):
        W1 = Wt[:, :, a, b, c]
        wmain[0:64, p, :] = W1[:, 0:128]
        wrem[0:64, p, :] = W1[:, 128:144]
        if K == 128:
            if sel == 0:
                a2, b2, c2 = a, b, c + 1
            elif sel == 1:
                a2, b2, c2 = a + 1, b, c
            else:
                a2, b2, c2 = a, b + 1, c
            W2 = Wt[:, :, a2, b2, c2]
            wmain[64:128, p, :] = W2[:, 0:128]
            wrem[64:128, p, :] = W2[:, 128:144]
    return (wmain.astype(BF16).reshape(128, -1),
            wrem.astype(BF16).reshape(128, -1))


def shard_x(x):
    """x (2,64,40,40,40) f32 -> per-core padded bf16 [64, LSTORE]."""
    xbf = np.ascontiguousarray(x).astype(BF16)
    shards = []
    for core in range(NCORES):
        b, chunk = divmod(core, 4)
        d0 = chunk * SLABS
        xp = np.zeros((64, DP, HP, WP), BF16)
        lo = max(0, d0 - 2)
        hi = min(S, d0 + SLABS + 2)
        xp[:, lo - (d0 - 2):hi - (d0 - 2), 2:2 + S, 2:2 + S] = xbf[b, :, lo:hi]
        flat = np.zeros((64, LSTORE), BF16)
        flat[:, :LFLAT] = xp.reshape(64, LFLAT)
        shards.append(flat)
    return shards


# ----------------------------------------------------------------------------
# bass program (built & compiled once per process)
# ----------------------------------------------------------------------------
_PROG = None


def _build_program():
    import concourse.bass as bass  # noqa: F401
    import concourse.bacc as bacc
    import concourse.mybir as mybir
    import concourse.tile as tile

    nc = bacc.Bacc("TRN2", target_bir_lowering=False, debug=False,
                   enable_asserts=False, num_devices=NCORES)
    bf = mybir.dt.bfloat16
    f32 = mybir.dt.float32

    xpad = nc.dram_tensor("xpad", [64, LSTORE], bf, kind="ExternalInput").ap()
    wmain_d = nc.dram_tensor("wmain", [128, NPAIR * 128], bf, kind="ExternalInput").ap()
    wrem_d = nc.dram_tensor("wrem", [128, NPAIR * 16], bf, kind="ExternalInput").ap()
    out_d = nc.dram_tensor("out", [DIM_OUT, SLABS, S, S], f32, kind="ExternalOutput").ap()
    warm_d = nc.dram_tensor("warm", [128, 16], f32, kind="ExternalOutput").ap()

    with tile.TileContext(nc) as tc:
        with tc.tile_pool(name="xpool", bufs=1) as xpool, \
             tc.tile_pool(name="wpool", bufs=1) as wpool, \
             tc.tile_pool(name="opool", bufs=3) as opool, \
             tc.tile_pool(name="pmain", bufs=4, space="PSUM") as pmain_pool, \
             tc.tile_pool(name="prem", bufs=3, space="PSUM") as prem_pool:

            # HAM warm-up: ~4us of dependency-free matmuls on zeroed scratch
            # run during the initial DMA fill (PE is otherwise idle), so the
            # real matmul stream starts at 2.4 GHz instead of 1.2 GHz.
            wscr = wpool.tile([128, 640], bf)
            nc.vector.memset(wscr, 0)
            pwarm = pmain_pool.tile([128, 512], f32, tag="pwarm", bufs=1)
            for _ in range(10):
                nc.tensor.matmul(pwarm, wscr[:, 0:128], wscr[:, 128:640],
                                 start=True, stop=True)
            wout = opool.tile([128, 16], f32, tag="wout", bufs=1)
            nc.vector.tensor_scalar_mul(wout, pwarm[:, 0:16], 1.0)
            nc.sync.dma_start(out=warm_d, in_=wout)

            xA = xpool.tile([128, ROWS, WP], bf)
            xB = xpool.tile([128, ROWS, WP], bf)
            xC = xpool.tile([128, ROWS, WP], bf)
            Wm = wpool.tile([128, NPAIR, 128], bf)
            Wr = wpool.tile([128, NPAIR, 16], bf)

            Wmf = Wm.rearrange("p a b -> p (a b)")

            xAf = xA.rearrange("p r c -> p (r c)")
            xBf = xB.rearrange("p r c -> p (r c)")
            xCf = xC.rearrange("p r c -> p (r c)")
            XTENS = (xA, xB, xC)

            def load_slice(xf, s, shift):
                sl = slice(s * SLICE, (s + 1) * SLICE)
                nc.sync.dma_start(out=xf[0:64, sl], in_=xpad[:, sl])
                nc.sync.dma_start(out=xf[64:128, sl],
                                  in_=xpad[:, s * SLICE + shift:(s + 1) * SLICE + shift])

            def load_w_chunk(p0, p1):
                nc.sync.dma_start(out=Wmf[:, p0 * 128:p1 * 128],
                                  in_=wmain_d[:, p0 * 128:p1 * 128])

            # need-ordered loads: slab-0 critical set first, weight DMA
            # chunked so the first matmuls (A-pairs with a=0, emitted first
            # in each accumulation group) can start after ~1MB of DMA.
            # used dst slices: A 0..13, B 0..11, C 4..13.
            load_w_chunk(0, 6)
            load_slice(xAf, 0, 1)
            load_w_chunk(6, 16)
            load_slice(xAf, 1, 1)
            load_w_chunk(16, 32)
            load_slice(xAf, 2, 1)
            load_w_chunk(32, 48)
            load_slice(xAf, 3, 1)
            load_w_chunk(48, NPAIR)
            load_slice(xAf, 4, 1)
            for s in range(4):
                load_slice(xBf, s, SLICE)
            load_slice(xCf, 4, HP)
            nc.sync.dma_start(out=Wr.rearrange("p a b -> p (a b)"), in_=wrem_d)
            for k in range(1, SLABS):
                load_slice(xAf, k + 4, 1)
                if k + 3 < 12:
                    load_slice(xBf, k + 3, SLICE)
                load_slice(xCf, k + 4, HP)

            def emit_main_tile(d, h0, nr):
                pm = pmain_pool.tile([128, nr, S], f32, tag="pm")
                for i, (sel, a, b, c, K) in enumerate(PAIRS):
                    r0 = (d + a) * HP + h0 + b
                    rhs = XTENS[sel][0:K, r0:r0 + nr, c:c + S]
                    nc.tensor.matmul(pm, Wm[0:K, i, :], rhs,
                                     start=(i == 0), stop=(i == NPAIR - 1))
                ot = opool.tile([128, nr, S], f32, tag="ot")
                nc.vector.tensor_scalar_mul(ot, pm, 0.1)
                nc.sync.dma_start(out=out_d[0:128, d, h0:h0 + nr, :], in_=ot)

            def emit_rem(d):
                # remainder couts 128..143, 4-way col-tiled (10 H-rows/group)
                pr = prem_pool.tile([128, 10, S], f32, tag="pr")
                for i, (sel, a, b, c, K) in enumerate(PAIRS):
                    for q in range(REM_Q):
                        r0 = (d + a) * HP + 10 * q + b
                        rhs = XTENS[sel][0:K, r0:r0 + 10, c:c + S]
                        nc.tensor.matmul(pr[32 * q:32 * q + 16, :, :],
                                         Wr[0:K, i, :], rhs,
                                         start=(i == 0), stop=(i == NPAIR - 1),
                                         tile_position=(0, 32 * q))
                orem = opool.tile([128, 10, S], f32, tag="orem")
                for q in range(REM_Q):
                    nc.vector.tensor_scalar_mul(orem[32 * q:32 * q + 16],
                                                pr[32 * q:32 * q + 16], 0.1)
                    nc.sync.dma_start(out=out_d[128:144, d, 10 * q:10 * q + 10, :],
                                      in_=orem[32 * q:32 * q + 16])

            for d in range(SLABS):
                for h0, nr in H_TILES[:-1]:
                    emit_main_tile(d, h0, nr)
                emit_rem(d)
                h0, nr = H_TILES[-1]
                emit_main_tile(d, h0, nr)

    nc.compile()
    return nc


def _get_program():
    global _PROG
    if _PROG is None:
        _PROG = _build_program()
    return _PROG


# ----------------------------------------------------------------------------
# entry points
# ----------------------------------------------------------------------------

def run_on_hw(inputs, trace=False, tmpdir=None):
    from concourse.bass_utils import run_bass_kernel_spmd

    nc = _get_program()
    ker = build_kernel_eff(inputs['w_sc0'], inputs['w_sc1'], inputs['w_tp'])
    wmain, wrem = pack_weights(ker)
    shards = shard_x(np.asarray(inputs['x']))
    in_maps = [{"xpad": shards[c], "wmain": wmain, "wrem": wrem}
               for c in range(NCORES)]
    res = run_bass_kernel_spmd(nc, in_maps, list(range(NCORES)),
                               trace=trace, tmpdir=tmpdir)
    out = np.zeros((BATCH, DIM_OUT, S, S, S), np.float32)
    for core in range(NCORES):
        b, chunk = divmod(core, 4)
        d0 = chunk * SLABS
        out[b, :, d0:d0 + SLABS] = res.results[core]["out"]
    return out, res


def kernel(x, w_sc0, w_sc1, w_tp):
    out, _ = run_on_hw({'x': x, 'w_sc0': w_sc0, 'w_sc1': w_sc1, 'w_tp': w_tp})
    return out
